# revision 1
# baseline (speedup 1.0000x reference)
"""Deformable-conv (DCNv1) Trainium2 Bass kernel, 8-way sharded.

Shapes (hardcoded from the problem spec):
  x  [2, 64, 128, 128] f32   input image
  Wp [18, 64, 3, 3]    f32   offset-conv weights (2*9 offset channels)
  bp [18]              f32   offset-conv bias
  Wc [64, 64, 3, 3]    f32   final conv weights (stride-3 over unfolded samples)
  out [2, 64, 128, 128] f32

Sharding: 8 cores; core k handles batch k//4, output rows (k%4)*32 .. +32.

Per-core pipeline (bf16 data path, f32 offset/index math):
  1. offset conv: 9 accumulating matmuls per output row (lhsT=Wp tap [64c,18m],
     rhs=x row window [64c,128w]) -> PSUM [(4j x 32-strided) part, 128 w],
     PE-transpose -> offsets [128 w, 4j*32] + bias.
  2. sampling positions px/py, clip, floor, fractions fr/fc/frc, gather index
     idx = floor_row*130 + floor_col, all on DVE in f32.
  3. one indirect-DMA gather per 4-row group: 128*36 rows x 256 bf16 from a
     host-prepacked difference table xt[16900, 256] where each padded pixel's
     row is [a | b=right-a | c=down-a | d=a+diag-right-down] x 64 channels.
  4. bilinear combine x_off = a + fc*b + fr*c + fr*fc*d as 3 chained
     scalar_tensor_tensor ops (per-partition scalar = that pixel's fraction).
  5. PE-transpose [128w, 2n*64c] tiles -> [(2n,c), w], final contraction
     accumulates 5 n-chunks into PSUM [64o, 128w]; DMA rows out.

Clipping exactness: the reference clips corner indices into [0,129] of the
zero-padded image, so every clipped corner reads a zero; unclipped fractions
with clipped indices reproduce it exactly. Table right/down neighbors at
index 129 are zero-extended (pad to 131), matching the reference's clip.
"""

import numpy as np
import ml_dtypes

import concourse.bacc as bacc
import concourse.bass as bass
import concourse.tile as tile
from concourse import mybir
from concourse.bass_utils import run_bass_kernel_spmd
from concourse.masks import make_identity

B, C, H, W, OUTC = 2, 64, 128, 128, 64
KS, N = 3, 9
PADH = H + 2  # 130
TROWS = PADH * PADH  # 16900 table rows
ROWS_PER_CORE = 32
NGROUP = 8          # groups of 4 output rows per core
JROWS = 4
NSAMP = JROWS * N   # 36 samples per w-pixel per group
XS_ROWS = ROWS_PER_CORE + 2

F32 = mybir.dt.float32
BF16 = mybir.dt.bfloat16
I32 = mybir.dt.int32
OP = mybir.AluOpType

_CACHE = {}


def _build_nc():
    nc = bacc.Bacc("TRN2", target_bir_lowering=False, debug=False)

    xt = nc.dram_tensor("xt", [TROWS, 4 * C], BF16, kind="ExternalInput")
    xs = nc.dram_tensor("xs", [C, XS_ROWS * PADH], BF16, kind="ExternalInput")
    wp = nc.dram_tensor("wp", [C, N * 32], BF16, kind="ExternalInput")
    bpt = nc.dram_tensor("bpt", [128, 128], F32, kind="ExternalInput")
    cpx = nc.dram_tensor("cpx", [128, NSAMP], F32, kind="ExternalInput")
    cpy = nc.dram_tensor("cpy", [128, NSAMP], F32, kind="ExternalInput")
    wc2 = nc.dram_tensor("wc2", [2 * C, 4 * OUTC], BF16, kind="ExternalInput")
    wc8 = nc.dram_tensor("wc8", [C, OUTC], BF16, kind="ExternalInput")
    sel = nc.dram_tensor("sel", [16, 128], F32, kind="ExternalInput")
    out = nc.dram_tensor("out", [OUTC, ROWS_PER_CORE * W], F32, kind="ExternalOutput")

    with tile.TileContext(nc) as tc:
        _emit(tc, nc, xt, xs, wp, bpt, cpx, cpy, wc2, wc8, sel, out)
    nc.compile()
    return nc


def _emit(tc, nc, xt, xs, wp, bpt, cpx, cpy, wc2, wc8, sel, out):
    from contextlib import ExitStack

    with ExitStack() as ctx:
        const = ctx.enter_context(tc.tile_pool(name="const", bufs=1))
        sb = ctx.enter_context(tc.tile_pool(name="sb", bufs=2))
        smalls = ctx.enter_context(tc.tile_pool(name="smalls", bufs=2))
        tmp = ctx.enter_context(tc.tile_pool(name="tmp", bufs=4))
        gpool = ctx.enter_context(tc.tile_pool(name="gpool", bufs=2))
        ps_conv = ctx.enter_context(tc.tile_pool(name="ps_conv", bufs=1, space="PSUM"))
        ps_off = ctx.enter_context(tc.tile_pool(name="ps_off", bufs=1, space="PSUM"))
        ps_xot = ctx.enter_context(tc.tile_pool(name="ps_xot", bufs=1, space="PSUM"))
        ps_o = ctx.enter_context(tc.tile_pool(name="ps_o", bufs=1, space="PSUM"))
        ps_idx = ctx.enter_context(tc.tile_pool(name="ps_idx", bufs=1, space="PSUM"))

        # ---- constants resident in SBUF ----
        ident = const.tile([128, 128], BF16)
        make_identity(nc, ident[:])
        identf = const.tile([128, 128], F32)
        make_identity(nc, identf[:])
        xs_sb = const.tile([C, XS_ROWS * PADH], BF16)
        nc.sync.dma_start(xs_sb[:], xs[:])
        wp_sb = const.tile([C, N * 32], BF16)
        nc.sync.dma_start(wp_sb[:], wp[:])
        bpt_sb = const.tile([128, 128], F32)
        nc.sync.dma_start(bpt_sb[:], bpt[:])
        cpx_sb = const.tile([128, NSAMP], F32)
        nc.sync.dma_start(cpx_sb[:], cpx[:])
        cpy_sb = const.tile([128, NSAMP], F32)
        nc.sync.dma_start(cpy_sb[:], cpy[:])
        wc2_sb = const.tile([2 * C, 4 * OUTC], BF16)
        nc.sync.dma_start(wc2_sb[:], wc2[:])
        wc8_sb = const.tile([C, OUTC], BF16)
        nc.sync.dma_start(wc8_sb[:], wc8[:])
        sel_sb = const.tile([16, 128], F32)
        nc.sync.dma_start(sel_sb[:], sel[:])

        for g in range(NGROUP):
            # ---- 1. offset conv for 4 rows -> PSUM [(4j x 32), 128 w] ----
            conv_ps = ps_conv.tile([128, W], F32)
            for j in range(JROWS):
                row = JROWS * g + j  # local row; padded source row = row + ti
                for t in range(N):
                    ti, tj = t // 3, t % 3
                    base = (row + ti) * PADH + tj
                    nc.tensor.matmul(
                        conv_ps[32 * j:32 * j + 32, :],
                        lhsT=wp_sb[:, t * 32:(t + 1) * 32],
                        rhs=xs_sb[:, base:base + W],
                        start=(t == 0),
                        stop=(t == N - 1),
                        tile_position=(0, 32 * j),
                    )
            conv_sb = smalls.tile([128, W], F32)
            nc.vector.tensor_copy(conv_sb[:], conv_ps[:])
            # transpose -> [128 w, 4j*32] then + bias
            offT_ps = ps_off.tile([128, 128], F32)
            nc.tensor.transpose(offT_ps[:], conv_sb[:], identf[:])
            offs = smalls.tile([128, 128], F32)
            nc.vector.tensor_tensor(offs[:], offT_ps[:], bpt_sb[:], OP.add)

            # ---- 2. positions / fractions / gather indices (f32) ----
            offs3 = offs[:].rearrange("p (j s) -> p j s", j=JROWS)
            cpx3 = cpx_sb[:].rearrange("p (j n) -> p j n", j=JROWS)
            cpy3 = cpy_sb[:].rearrange("p (j n) -> p j n", j=JROWS)

            px = smalls.tile([128, NSAMP], F32)
            px3 = px[:].rearrange("p (j n) -> p j n", j=JROWS)
            # px = offx + (4g+1) + cpx   (cpx carries h0 + pnx[n] + j from host)
            nc.vector.scalar_tensor_tensor(
                px3, offs3[:, :, 0:N], float(JROWS * g + 1), cpx3, OP.add, OP.add)
            py = smalls.tile([128, NSAMP], F32)
            py3 = py[:].rearrange("p (j n) -> p j n", j=JROWS)
            nc.vector.tensor_tensor(py3, offs3[:, :, N:2 * N], cpy3, OP.add)

            nc.vector.tensor_scalar(px[:], px[:], 129.0, 0.0, OP.min, OP.max)
            nc.vector.tensor_scalar(py[:], py[:], 129.0, 0.0, OP.min, OP.max)

            def floor_of(src):
                fi = tmp.tile([128, NSAMP], I32, name="fi")
                nc.vector.tensor_copy(fi[:], src[:])
                ff = smalls.tile([128, NSAMP], F32, name="ff")
                nc.vector.tensor_copy(ff[:], fi[:])
                gt = tmp.tile([128, NSAMP], F32, name="gt")
                nc.vector.tensor_tensor(gt[:], ff[:], src[:], OP.is_gt)
                nc.vector.tensor_tensor(ff[:], ff[:], gt[:], OP.subtract)
                return ff

            flr = floor_of(px)
            flc = floor_of(py)

            fr = smalls.tile([128, NSAMP], F32)
            nc.vector.tensor_tensor(fr[:], px[:], flr[:], OP.subtract)
            fc = smalls.tile([128, NSAMP], F32)
            nc.vector.tensor_tensor(fc[:], py[:], flc[:], OP.subtract)
            frc = smalls.tile([128, NSAMP], F32)
            nc.vector.tensor_tensor(frc[:], fr[:], fc[:], OP.mult)

            idx_f = smalls.tile([128, NSAMP], F32)
            nc.vector.scalar_tensor_tensor(
                idx_f[:], flr[:], float(PADH), flc[:], OP.mult, OP.add)

            # ---- 3. wrap indices for dma_gather: sample i=(t*128+16u+q) sits
            # at wrapped position [q, t*8+u]; build via two PE transposes and a
            # selector matmul that also replicates across the 8 Q7 cores.
            idxT_ps = ps_idx.tile([NSAMP, 128], F32)
            nc.tensor.transpose(idxT_ps[:], idx_f[:], identf[:])
            idxT = smalls.tile([NSAMP, 128], F32)
            nc.vector.tensor_copy(idxT[:], idxT_ps[:])
            w16 = smalls.tile([16, 8 * NSAMP], F32)
            w16v = w16[:].rearrange("p (t u) -> p t u", u=8)
            for u in range(8):
                t2_ps = ps_idx.tile([16, NSAMP], F32, name="t2_ps")
                nc.tensor.transpose(
                    t2_ps[:], idxT[:, 16 * u:16 * u + 16], identf[0:NSAMP, 0:NSAMP])
                nc.vector.tensor_copy(w16v[:, :, u], t2_ps[:])
            rep_ps = ps_idx.tile([128, 8 * NSAMP], F32)
            nc.tensor.matmul(rep_ps[:], lhsT=sel_sb[:], rhs=w16[:], start=True, stop=True)
            idx16 = smalls.tile([128, 8 * NSAMP], mybir.dt.int16)
            nc.vector.tensor_copy(idx16[:], rep_ps[:])

            gath = gpool.tile([128, NSAMP, 4 * C], BF16)
            for j in range(JROWS):
                nc.gpsimd.dma_gather(
                    gath[:, j * N:(j + 1) * N, :], xt[:],
                    idx16[:, j * 8 * N:(j + 1) * 8 * N],
                    num_idxs=128 * N, num_idxs_reg=128 * N,
                    elem_size=4 * C, elem_step=4 * C,
                    single_packet=False,
                )

            # ---- 4. bilinear combine ----
            xo = sb.tile([128, JROWS, N * C], BF16)
            for j in range(JROWS):
                for n in range(N):
                    s = j * N + n
                    col = (n // 2) * 2 * C + (n % 2) * C if n < 8 else 8 * C
                    a_ = gath[:, s, 0:C]
                    b_ = gath[:, s, C:2 * C]
                    c_ = gath[:, s, 2 * C:3 * C]
                    d_ = gath[:, s, 3 * C:4 * C]
                    t1 = tmp.tile([128, C], BF16, name="t1")
                    nc.vector.scalar_tensor_tensor(
                        t1[:], b_, fc[:, s:s + 1], a_, OP.mult, OP.add)
                    t2 = tmp.tile([128, C], BF16, name="t2")
                    nc.vector.scalar_tensor_tensor(
                        t2[:], c_, fr[:, s:s + 1], t1[:], OP.mult, OP.add)
                    nc.vector.scalar_tensor_tensor(
                        xo[:, j, col:col + C], d_, frc[:, s:s + 1], t2[:],
                        OP.mult, OP.add)

            # ---- 5. transpose + final matmuls ----
            out_sb = sb.tile([OUTC, JROWS * W], F32)
            for j in range(JROWS):
                xot_ps = ps_xot.tile([128, 4 * 128], BF16)
                for q in range(4):
                    nc.tensor.transpose(
                        xot_ps[:, q * 128:(q + 1) * 128],
                        xo[:, j, q * 2 * C:(q + 1) * 2 * C],
                        ident[:],
                    )
                xot8_ps = ps_off.tile([C, 128], BF16, name="xot8_ps")
                nc.tensor.transpose(xot8_ps[:], xo[:, j, 8 * C:9 * C], ident[:])
                xot = sb.tile([128, 4 * 128], BF16, name="xot")
                nc.vector.tensor_copy(xot[:], xot_ps[:])
                xot8 = sb.tile([C, 128], BF16, name="xot8")
                nc.vector.tensor_copy(xot8[:], xot8_ps[:])

                o_ps = ps_o.tile([OUTC, W], F32)
                for q in range(4):
                    nc.tensor.matmul(
                        o_ps[:],
                        lhsT=wc2_sb[:, q * OUTC:(q + 1) * OUTC],
                        rhs=xot[:, q * 128:(q + 1) * 128],
                        start=(q == 0),
                        stop=False,
                    )
                nc.tensor.matmul(
                    o_ps[:], lhsT=wc8_sb[:], rhs=xot8[:], start=False, stop=True)
                nc.vector.tensor_copy(out_sb[:, j * W:(j + 1) * W], o_ps[:])
            nc.sync.dma_start(out[:, g * JROWS * W:(g + 1) * JROWS * W], out_sb[:])


def _host_prep(x, Wp, bp, Wc):
    x = np.asarray(x, np.float32)
    Wp = np.asarray(Wp, np.float32)
    bp = np.asarray(bp, np.float32)
    Wc = np.asarray(Wc, np.float32)
    bf16 = ml_dtypes.bfloat16

    # difference tables per batch
    tables = []
    for b in range(B):
        xp2 = np.pad(x[b], ((0, 0), (1, 2), (1, 2)))  # [C, 131, 131]
        a = xp2[:, :PADH, :PADH]
        r = xp2[:, :PADH, 1:PADH + 1]
        d = xp2[:, 1:PADH + 1, :PADH]
        dg = xp2[:, 1:PADH + 1, 1:PADH + 1]
        comp = np.stack([a, r - a, d - a, a + dg - r - d], axis=0)  # [4,C,130,130]
        t = comp.transpose(2, 3, 0, 1).reshape(TROWS, 4 * C)
        tables.append(np.ascontiguousarray(t.astype(bf16)))

    # wp[c, t*32+m] = Wp[m, c, t//3, t%3] for m<18, zero-padded to 32
    wp_r = np.zeros((C, N, 32), np.float32)
    wp_r[:, :, :2 * N] = Wp.reshape(2 * N, C, N).transpose(1, 2, 0)
    wp_r = np.ascontiguousarray(wp_r.reshape(C, N * 32).astype(bf16))

    bpt = np.zeros((128, 128), np.float32)
    for j in range(JROWS):
        bpt[:, 32 * j:32 * j + 2 * N] = bp[None, :]

    nidx = np.arange(N)
    pnx = (nidx // 3 - 1).astype(np.float32)
    pny = (nidx % 3 - 1).astype(np.float32)
    p = np.arange(128, dtype=np.float32)
    cpx0 = np.zeros((128, NSAMP), np.float32)
    cpy = np.zeros((128, NSAMP), np.float32)
    for j in range(JROWS):
        cpx0[:, j * N:(j + 1) * N] = pnx[None, :] + j
        cpy[:, j * N:(j + 1) * N] = pny[None, :] + (p[:, None] + 1.0)

    wcf = Wc.reshape(OUTC, C, N)
    wc2 = np.zeros((2 * C, 4 * OUTC), np.float32)
    for q in range(4):
        for s_ in range(2):
            wc2[s_ * C:(s_ + 1) * C, q * OUTC:(q + 1) * OUTC] = wcf[:, :, 2 * q + s_].T
    wc8 = np.ascontiguousarray(wcf[:, :, 8].T)

    sel_m = (np.arange(128)[None, :] % 16 == np.arange(16)[:, None]).astype(np.float32)

    xp1 = [np.pad(x[b], ((0, 0), (1, 1), (1, 1))) for b in range(B)]

    in_maps = []
    for k in range(8):
        bk, h0 = k // 4, (k % 4) * ROWS_PER_CORE
        xs_slice = xp1[bk][:, h0:h0 + XS_ROWS, :].reshape(C, XS_ROWS * PADH)
        in_maps.append({
            "xt": tables[bk],
            "xs": np.ascontiguousarray(xs_slice.astype(bf16)),
            "wp": wp_r,
            "bpt": bpt,
            "cpx": cpx0 + np.float32(h0),
            "cpy": cpy,
            "wc2": np.ascontiguousarray(wc2.astype(bf16)),
            "wc8": wc8.astype(bf16),
            "sel": sel_m,
        })
    return in_maps


def kernel(x, Wp, bp, Wc):
    if "nc" not in _CACHE:
        _CACHE["nc"] = _build_nc()
    nc = _CACHE["nc"]
    in_maps = _host_prep(x, Wp, bp, Wc)
    res = run_bass_kernel_spmd(nc, in_maps, list(range(8)))
    _CACHE["exec_time_ns"] = res.exec_time_ns
    _CACHE["results"] = res
    out = np.zeros((B, OUTC, H, W), np.float32)
    for k in range(8):
        bk, h0 = k // 4, (k % 4) * ROWS_PER_CORE
        out[bk, :, h0:h0 + ROWS_PER_CORE, :] = res.results[k]["out"].reshape(
            OUTC, ROWS_PER_CORE, W)
    return out



# revision 9
# speedup vs baseline: 1.9689x; 1.9689x over previous
"""Deformable-conv (DCNv1) Trainium2 Bass kernel, 8-way sharded.

Shapes (hardcoded from the problem spec):
  x  [2, 64, 128, 128] f32   input image
  Wp [18, 64, 3, 3]    f32   offset-conv weights (2*9 offset channels)
  bp [18]              f32   offset-conv bias
  Wc [64, 64, 3, 3]    f32   final conv weights (stride-3 over unfolded samples)
  out [2, 64, 128, 128] f32

Sharding: 8 cores; core k handles batch k//4, output rows (k%4)*32 .. +32.

Per-core pipeline (bf16 data path, f32 offset/index math):
  1. offset conv: 9 accumulating matmuls per output row (lhsT=Wp tap [64c,18m],
     rhs=x row window [64c,128w]) -> PSUM [(4j x 32-strided) part, 128 w],
     PE-transpose -> offsets [128 w, 4j*32] + bias.
  2. sampling positions px/py, clip, floor, fractions fr/fc/frc, gather index
     idx = floor_row*130 + floor_col, all on DVE in f32.
  3. one indirect-DMA gather per 4-row group: 128*36 rows x 256 bf16 from a
     host-prepacked difference table xt[16900, 256] where each padded pixel's
     row is [a | b=right-a | c=down-a | d=a+diag-right-down] x 64 channels.
  4. bilinear combine x_off = a + fc*b + fr*c + fr*fc*d as 3 chained
     scalar_tensor_tensor ops (per-partition scalar = that pixel's fraction).
  5. PE-transpose [128w, 2n*64c] tiles -> [(2n,c), w], final contraction
     accumulates 5 n-chunks into PSUM [64o, 128w]; DMA rows out.

Clipping exactness: the reference clips corner indices into [0,129] of the
zero-padded image, so every clipped corner reads a zero; unclipped fractions
with clipped indices reproduce it exactly. Table right/down neighbors at
index 129 are zero-extended (pad to 131), matching the reference's clip.
"""

import numpy as np
import ml_dtypes

import concourse.bacc as bacc
import concourse.bass as bass
import concourse.tile as tile
from concourse import mybir
from concourse.bass_utils import run_bass_kernel_spmd
from concourse.masks import make_identity

B, C, H, W, OUTC = 2, 64, 128, 128, 64
KS, N = 3, 9
PADH = H + 2  # 130
TROWS = PADH * PADH  # 16900 table rows
ROWS_PER_CORE = 32
NGROUP = 8          # groups of 4 output rows per core
JROWS = 4
NSAMP = JROWS * N   # 36 samples per w-pixel per group
XS_ROWS = ROWS_PER_CORE + 2

F32 = mybir.dt.float32
BF16 = mybir.dt.bfloat16
I32 = mybir.dt.int32
OP = mybir.AluOpType

_CACHE = {}


def _ap(base, extra_off, dims):
    """Manual AP: keep base's partition dim, supply free dims [[stride,num],..]."""
    return bass.AP(base.tensor, base.offset + extra_off, [base.ap[0]] + dims)


def _build_nc():
    nc = bacc.Bacc("TRN2", target_bir_lowering=False, debug=False,
                   num_swdge_queues=4)

    xt = nc.dram_tensor("xt", [TROWS, 4 * C], BF16, kind="ExternalInput")
    xs = nc.dram_tensor("xs", [C, XS_ROWS * PADH], BF16, kind="ExternalInput")
    wp = nc.dram_tensor("wp", [C, N * 32], BF16, kind="ExternalInput")
    bpt = nc.dram_tensor("bpt", [128, 128], F32, kind="ExternalInput")
    cpx = nc.dram_tensor("cpx", [128, NSAMP], F32, kind="ExternalInput")
    cpy = nc.dram_tensor("cpy", [128, NSAMP], F32, kind="ExternalInput")
    wc2 = nc.dram_tensor("wc2", [2 * C, 4 * OUTC], BF16, kind="ExternalInput")
    wc8 = nc.dram_tensor("wc8", [C, OUTC], BF16, kind="ExternalInput")
    sel = nc.dram_tensor("sel", [16, 128], F32, kind="ExternalInput")
    out = nc.dram_tensor("out", [OUTC, ROWS_PER_CORE * W], F32, kind="ExternalOutput")

    with tile.TileContext(nc) as tc:
        _emit(tc, nc, xt, xs, wp, bpt, cpx, cpy, wc2, wc8, sel, out)
    nc.compile()
    return nc


def _emit(tc, nc, xt, xs, wp, bpt, cpx, cpy, wc2, wc8, sel, out):
    from contextlib import ExitStack

    with ExitStack() as ctx:
        const = ctx.enter_context(tc.tile_pool(name="const", bufs=1))
        sb = ctx.enter_context(tc.tile_pool(name="sb", bufs=2))
        smalls = ctx.enter_context(tc.tile_pool(name="smalls", bufs=2))
        tmp = ctx.enter_context(tc.tile_pool(name="tmp", bufs=4))
        gpool = ctx.enter_context(tc.tile_pool(name="gpool", bufs=2))
        ps_conv = ctx.enter_context(tc.tile_pool(name="ps_conv", bufs=1, space="PSUM"))
        ps_off = ctx.enter_context(tc.tile_pool(name="ps_off", bufs=1, space="PSUM"))
        ps_xot = ctx.enter_context(tc.tile_pool(name="ps_xot", bufs=1, space="PSUM"))
        ps_o = ctx.enter_context(tc.tile_pool(name="ps_o", bufs=1, space="PSUM"))
        ps_idx = ctx.enter_context(tc.tile_pool(name="ps_idx", bufs=1, space="PSUM"))

        # ---- constants resident in SBUF ----
        ident = const.tile([128, 128], BF16)
        make_identity(nc, ident[:])
        identf = const.tile([128, 128], F32)
        make_identity(nc, identf[:])
        xs_sb = const.tile([C, XS_ROWS * PADH], BF16)
        nc.sync.dma_start(xs_sb[:], xs[:])
        wp_sb = const.tile([C, N * 32], BF16)
        nc.sync.dma_start(wp_sb[:], wp[:])
        bpt_sb = const.tile([128, 128], F32)
        nc.sync.dma_start(bpt_sb[:], bpt[:])
        cpx_sb = const.tile([128, NSAMP], F32)
        nc.sync.dma_start(cpx_sb[:], cpx[:])
        cpy_sb = const.tile([128, NSAMP], F32)
        nc.sync.dma_start(cpy_sb[:], cpy[:])
        wc2_sb = const.tile([2 * C, 4 * OUTC], BF16)
        nc.sync.dma_start(wc2_sb[:], wc2[:])
        wc8_sb = const.tile([C, OUTC], BF16)
        nc.sync.dma_start(wc8_sb[:], wc8[:])
        sel_sb = const.tile([16, 128], F32)
        nc.sync.dma_start(sel_sb[:], sel[:])

        for g in range(NGROUP):
            # ---- 1. offset conv for 4 rows -> PSUM [(4j x 32), 128 w] ----
            conv_ps = ps_conv.tile([128, W], F32)
            for j in range(JROWS):
                row = JROWS * g + j  # local row; padded source row = row + ti
                for t in range(N):
                    ti, tj = t // 3, t % 3
                    base = (row + ti) * PADH + tj
                    nc.tensor.matmul(
                        conv_ps[32 * j:32 * j + 32, :],
                        lhsT=wp_sb[:, t * 32:(t + 1) * 32],
                        rhs=xs_sb[:, base:base + W],
                        start=(t == 0),
                        stop=(t == N - 1),
                        tile_position=(0, 32 * j),
                    )
            conv_sb = smalls.tile([128, W], F32)
            nc.vector.tensor_copy(conv_sb[:], conv_ps[:])
            # transpose -> [128 w, 4j*32] then + bias
            offT_ps = ps_off.tile([128, 128], F32)
            nc.tensor.transpose(offT_ps[:], conv_sb[:], identf[:])
            offs = smalls.tile([128, 128], F32)
            nc.vector.tensor_tensor(offs[:], offT_ps[:], bpt_sb[:], OP.add)

            # ---- 2. positions / fractions / gather indices (f32) ----
            offs3 = offs[:].rearrange("p (j s) -> p j s", j=JROWS)
            cpx3 = cpx_sb[:].rearrange("p (j n) -> p j n", j=JROWS)
            cpy3 = cpy_sb[:].rearrange("p (j n) -> p j n", j=JROWS)

            px = smalls.tile([128, NSAMP], F32)
            px3 = px[:].rearrange("p (j n) -> p j n", j=JROWS)
            # px = offx + (4g+1) + cpx   (cpx carries h0 + pnx[n] + j from host)
            nc.vector.scalar_tensor_tensor(
                px3, offs3[:, :, 0:N], float(JROWS * g + 1), cpx3, OP.add, OP.add)
            py = smalls.tile([128, NSAMP], F32)
            py3 = py[:].rearrange("p (j n) -> p j n", j=JROWS)
            nc.vector.tensor_tensor(py3, offs3[:, :, N:2 * N], cpy3, OP.add)

            nc.vector.tensor_scalar(px[:], px[:], 129.0, 0.0, OP.min, OP.max)
            nc.vector.tensor_scalar(py[:], py[:], 129.0, 0.0, OP.min, OP.max)

            def floor_of(src):
                fi = tmp.tile([128, NSAMP], I32, name="fi")
                nc.vector.tensor_copy(fi[:], src[:])
                ff = smalls.tile([128, NSAMP], F32, name="ff")
                nc.vector.tensor_copy(ff[:], fi[:])
                gt = tmp.tile([128, NSAMP], F32, name="gt")
                nc.vector.tensor_tensor(gt[:], ff[:], src[:], OP.is_gt)
                nc.vector.tensor_tensor(ff[:], ff[:], gt[:], OP.subtract)
                return ff

            flr = floor_of(px)
            flc = floor_of(py)

            fr = smalls.tile([128, NSAMP], F32)
            nc.vector.tensor_tensor(fr[:], px[:], flr[:], OP.subtract)
            fc = smalls.tile([128, NSAMP], F32)
            nc.vector.tensor_tensor(fc[:], py[:], flc[:], OP.subtract)
            frc = smalls.tile([128, NSAMP], F32)
            nc.vector.tensor_tensor(frc[:], fr[:], fc[:], OP.mult)

            idx_f = smalls.tile([128, NSAMP], F32)
            nc.vector.scalar_tensor_tensor(
                idx_f[:], flr[:], float(PADH), flc[:], OP.mult, OP.add)

            # ---- 3. wrap indices for dma_gather: sample i=(t*128+16u+q) sits
            # at wrapped position [q, t*8+u]; build via two PE transposes and a
            # selector matmul that also replicates across the 8 Q7 cores.
            idxT_ps = ps_idx.tile([NSAMP, 128], F32)
            nc.tensor.transpose(idxT_ps[:], idx_f[:], identf[:])
            idxT = smalls.tile([NSAMP, 128], F32)
            nc.vector.tensor_copy(idxT[:], idxT_ps[:])
            w16 = smalls.tile([16, 8 * NSAMP], F32)
            w16v = w16[:].rearrange("p (t u) -> p t u", u=8)
            for u in range(8):
                t2_ps = ps_idx.tile([16, NSAMP], F32, name="t2_ps")
                nc.tensor.transpose(
                    t2_ps[:], idxT[:, 16 * u:16 * u + 16], identf[0:NSAMP, 0:NSAMP])
                nc.vector.tensor_copy(w16v[:, :, u], t2_ps[:])
            rep_ps = ps_idx.tile([128, 8 * NSAMP], F32)
            nc.tensor.matmul(rep_ps[:], lhsT=sel_sb[:], rhs=w16[:], start=True, stop=True)
            idx16 = smalls.tile([128, 8 * NSAMP], mybir.dt.int16)
            nc.vector.tensor_copy(idx16[:], rep_ps[:])

            gath = gpool.tile([128, NSAMP, 4 * C], BF16)
            for j in range(JROWS):
                nc.gpsimd.dma_gather(
                    gath[:, j * N:(j + 1) * N, :], xt[:],
                    idx16[:, j * 8 * N:(j + 1) * 8 * N],
                    num_idxs=128 * N, num_idxs_reg=128 * N,
                    elem_size=4 * C, elem_step=4 * C,
                    single_packet=False,
                    queue_num=j,
                )

            # ---- 4. bilinear combine, batched: 6 stt passes over the whole
            # group (DVE ISA allows at most 2 free dims, so the fraction
            # operand broadcasts over the full channel dim via one stride-0
            # dim; mult passes run 1x, add passes hit the packed fast path).
            # Note xo col for sample n is just n*C.
            gv = gath[:]
            dims_g = [[4 * C, NSAMP], [1, C]]
            A_ = _ap(gv, 0, dims_g)
            B_ = _ap(gv, C, dims_g)
            C_ = _ap(gv, 2 * C, dims_g)
            D_ = _ap(gv, 3 * C, dims_g)

            def fb(t):
                return _ap(t[:], 0, [[1, NSAMP], [0, C]])

            xo = sb.tile([128, JROWS, N * C], BF16)
            dims_x = [[C, NSAMP], [1, C]]
            xo3 = _ap(xo[:], 0, dims_x)
            t1 = tmp.tile([128, NSAMP * C], BF16, name="t1")
            t13 = _ap(t1[:], 0, dims_x)
            acc = tmp.tile([128, NSAMP * C], BF16, name="acc")
            acc3 = _ap(acc[:], 0, dims_x)
            nc.vector.scalar_tensor_tensor(t13, B_, 1.0, fb(fc), OP.mult, OP.mult)
            nc.vector.scalar_tensor_tensor(acc3, t13, 1.0, A_, OP.mult, OP.add)
            nc.vector.scalar_tensor_tensor(t13, C_, 1.0, fb(fr), OP.mult, OP.mult)
            nc.vector.scalar_tensor_tensor(acc3, t13, 1.0, acc3, OP.mult, OP.add)
            nc.vector.scalar_tensor_tensor(t13, D_, 1.0, fb(frc), OP.mult, OP.mult)
            nc.vector.scalar_tensor_tensor(xo3, t13, 1.0, acc3, OP.mult, OP.add)

            # ---- 5. transpose + final matmuls ----
            out_sb = sb.tile([OUTC, JROWS * W], F32)
            for j in range(JROWS):
                xot_ps = ps_xot.tile([128, 4 * 128], BF16)
                for q in range(4):
                    nc.tensor.transpose(
                        xot_ps[:, q * 128:(q + 1) * 128],
                        xo[:, j, q * 2 * C:(q + 1) * 2 * C],
                        ident[:],
                    )
                xot8_ps = ps_off.tile([C, 128], BF16, name="xot8_ps")
                nc.tensor.transpose(xot8_ps[:], xo[:, j, 8 * C:9 * C], ident[:])
                xot = sb.tile([128, 4 * 128], BF16, name="xot")
                nc.vector.tensor_copy(xot[:], xot_ps[:])
                xot8 = sb.tile([C, 128], BF16, name="xot8")
                nc.vector.tensor_copy(xot8[:], xot8_ps[:])

                o_ps = ps_o.tile([OUTC, W], F32)
                for q in range(4):
                    nc.tensor.matmul(
                        o_ps[:],
                        lhsT=wc2_sb[:, q * OUTC:(q + 1) * OUTC],
                        rhs=xot[:, q * 128:(q + 1) * 128],
                        start=(q == 0),
                        stop=False,
                    )
                nc.tensor.matmul(
                    o_ps[:], lhsT=wc8_sb[:], rhs=xot8[:], start=False, stop=True)
                nc.vector.tensor_copy(out_sb[:, j * W:(j + 1) * W], o_ps[:])
            nc.sync.dma_start(out[:, g * JROWS * W:(g + 1) * JROWS * W], out_sb[:])


def _host_prep(x, Wp, bp, Wc):
    x = np.asarray(x, np.float32)
    Wp = np.asarray(Wp, np.float32)
    bp = np.asarray(bp, np.float32)
    Wc = np.asarray(Wc, np.float32)
    bf16 = ml_dtypes.bfloat16

    # difference tables per batch
    tables = []
    for b in range(B):
        xp2 = np.pad(x[b], ((0, 0), (1, 2), (1, 2)))  # [C, 131, 131]
        a = xp2[:, :PADH, :PADH]
        r = xp2[:, :PADH, 1:PADH + 1]
        d = xp2[:, 1:PADH + 1, :PADH]
        dg = xp2[:, 1:PADH + 1, 1:PADH + 1]
        comp = np.stack([a, r - a, d - a, a + dg - r - d], axis=0)  # [4,C,130,130]
        t = comp.transpose(2, 3, 0, 1).reshape(TROWS, 4 * C)
        tables.append(np.ascontiguousarray(t.astype(bf16)))

    # wp[c, t*32+m] = Wp[m, c, t//3, t%3] for m<18, zero-padded to 32
    wp_r = np.zeros((C, N, 32), np.float32)
    wp_r[:, :, :2 * N] = Wp.reshape(2 * N, C, N).transpose(1, 2, 0)
    wp_r = np.ascontiguousarray(wp_r.reshape(C, N * 32).astype(bf16))

    bpt = np.zeros((128, 128), np.float32)
    for j in range(JROWS):
        bpt[:, 32 * j:32 * j + 2 * N] = bp[None, :]

    nidx = np.arange(N)
    pnx = (nidx // 3 - 1).astype(np.float32)
    pny = (nidx % 3 - 1).astype(np.float32)
    p = np.arange(128, dtype=np.float32)
    cpx0 = np.zeros((128, NSAMP), np.float32)
    cpy = np.zeros((128, NSAMP), np.float32)
    for j in range(JROWS):
        cpx0[:, j * N:(j + 1) * N] = pnx[None, :] + j
        cpy[:, j * N:(j + 1) * N] = pny[None, :] + (p[:, None] + 1.0)

    wcf = Wc.reshape(OUTC, C, N)
    wc2 = np.zeros((2 * C, 4 * OUTC), np.float32)
    for q in range(4):
        for s_ in range(2):
            wc2[s_ * C:(s_ + 1) * C, q * OUTC:(q + 1) * OUTC] = wcf[:, :, 2 * q + s_].T
    wc8 = np.ascontiguousarray(wcf[:, :, 8].T)

    sel_m = (np.arange(128)[None, :] % 16 == np.arange(16)[:, None]).astype(np.float32)

    xp1 = [np.pad(x[b], ((0, 0), (1, 1), (1, 1))) for b in range(B)]

    in_maps = []
    for k in range(8):
        bk, h0 = k // 4, (k % 4) * ROWS_PER_CORE
        xs_slice = xp1[bk][:, h0:h0 + XS_ROWS, :].reshape(C, XS_ROWS * PADH)
        in_maps.append({
            "xt": tables[bk],
            "xs": np.ascontiguousarray(xs_slice.astype(bf16)),
            "wp": wp_r,
            "bpt": bpt,
            "cpx": cpx0 + np.float32(h0),
            "cpy": cpy,
            "wc2": np.ascontiguousarray(wc2.astype(bf16)),
            "wc8": wc8.astype(bf16),
            "sel": sel_m,
        })
    return in_maps


def kernel(x, Wp, bp, Wc):
    if "nc" not in _CACHE:
        _CACHE["nc"] = _build_nc()
    nc = _CACHE["nc"]
    in_maps = _host_prep(x, Wp, bp, Wc)
    res = run_bass_kernel_spmd(nc, in_maps, list(range(8)))
    _CACHE["exec_time_ns"] = res.exec_time_ns
    _CACHE["results"] = res
    out = np.zeros((B, OUTC, H, W), np.float32)
    for k in range(8):
        bk, h0 = k // 4, (k % 4) * ROWS_PER_CORE
        out[bk, :, h0:h0 + ROWS_PER_CORE, :] = res.results[k]["out"].reshape(
            OUTC, ROWS_PER_CORE, W)
    return out

